# revision 17
# baseline (speedup 1.0000x reference)
"""CrissCrossAttention Trainium2 kernel (v4).

Full inputs in, full output out. Data-parallel over batch across 8 cores
(B=16 -> 2 images per core). Per image (H=W=128, C=256, D=32):

  - x uploaded HOST-TRANSPOSED (channel-major) -> XT load is plain DMA,
    no XBAR transposes. Residual (x + gamma*bv) uploaded w-major.
  - q/k projected with a packed [128, 64] weight (M=64); bias applied in
    the PSUM drains (vector tensor_scalar / scalar ACT Identity).
  - V computed ONCE per image, SBUF-resident half-image tile
    [128h, 64w, 257] for the column branch; scattered to a w-major DRAM
    scratch (interleaved with U_h) for the row branch. Column 257 holds
    1/GAMMA so the epilogue scale is just reciprocal(S/gamma).
  - Column phase per 4-w group: 4 K=32 energy matmuls + one wide -120*I
    mask matmul into one PSUM bank, one exp, per-w aggregation against
    the resident V. U_h+denominator tiles scatter to DRAM.
  - Row phase: energies + aggregation + U_h merge (identity matmul), then
    per row: out = U*(gamma/S) + xres, fused scalar_tensor_tensor on
    vector for even rows, scalar ACT + gpsimd add for odd rows (gpsimd
    cannot read PSUM). Reciprocals issue first to shorten the PSUM-free
    chain.
  - PSUM: 6 single-bank tiles rotate through projections/V/aggregation;
    2 banks for energy groups. Software pipelining (A depth 2, B depth 3)
    keeps the PE fed to hold its fast p-state.
  - Output written bf16 w-major; host transposes back.
"""

import os
import sys

import numpy as np

try:
    import concourse  # noqa: F401
except ImportError:
    for p in ("/root/.axon_site/_ro/trn_rl_repo", "/opt/trn_rl_repo"):
        if os.path.isdir(p):
            sys.path.insert(0, p)
            break

import ml_dtypes

import concourse.bass as bass  # noqa: F401
import concourse.tile as tile
from concourse import bacc, mybir
from concourse.bass_utils import run_bass_kernel_spmd

BF16 = mybir.dt.bfloat16
F32 = mybir.dt.float32
AF = mybir.ActivationFunctionType
ALU = mybir.AluOpType

B, H, W, C, D = 16, 128, 128, 256, 32
NCORES = 8
BPC = B // NCORES  # images per core
HWPIX = H * W
GAMMA = 0.05
RGAMMA = 1.0 / GAMMA  # ones-column value -> denominators arrive as S/gamma
NEGBIG = -120.0  # diagonal mask offset; exp(e-120) underflows to 0
CU = C + 1  # v carries a 1/gamma column -> softmax denominator
WHALF = W // 2  # V kept resident for half the image's columns at a time


def build_program():
    nc = bacc.Bacc(
        "TRN2",
        target_bir_lowering=False,
        debug=False,
        num_devices=NCORES,
    )

    # channel-major x: [bi, chunk, cpart, pix] (pix h-major: p = h*W + w)
    xbf_t = nc.dram_tensor("xbf_t", [BPC, 2, 128, HWPIX], BF16, kind="ExternalInput").ap()
    # residual (x + gamma*bv), w-major pixels: p = w*H + h
    xres_w = nc.dram_tensor("xres_w", [BPC, HWPIX, C], BF16, kind="ExternalInput").ap()
    wqk_d = nc.dram_tensor("wqk_b", [2, 128, 2 * D], BF16, kind="ExternalInput").ap()
    wv_d = nc.dram_tensor("wv_b", [2, 128, C], BF16, kind="ExternalInput").ap()
    bqk_d = nc.dram_tensor("bqk_f", [2 * D, 1], F32, kind="ExternalInput").ap()
    eye_d = nc.dram_tensor("eye_b", [128, 128], BF16, kind="ExternalInput").ap()
    negi4_d = nc.dram_tensor("negi4_b", [128, 512], BF16, kind="ExternalInput").ap()
    # interleaved scratch, w-major pixels: [...,0,:] = U_h tiles, [...,1,:] = V
    uhv1_d = nc.dram_tensor("uhv1", [BPC, HWPIX, 2, CU], BF16, kind="Internal").ap()
    # w-major output; host transposes back
    out_d = nc.dram_tensor("out", [BPC, HWPIX, C], BF16, kind="ExternalOutput").ap()

    with tile.TileContext(nc) as tc:
        with (
            tc.tile_pool(name="const", bufs=1) as constp,
            tc.tile_pool(name="xt", bufs=1) as xtp,
            tc.tile_pool(name="qk", bufs=1) as qkp,
            tc.tile_pool(name="vres", bufs=1) as vresp,
            tc.tile_pool(name="ex", bufs=4) as ep,
            tc.tile_pool(name="ust", bufs=2) as usp,
            tc.tile_pool(name="ulvl", bufs=2) as ulp,
            tc.tile_pool(name="xr", bufs=2) as xrp,
            tc.tile_pool(name="ost", bufs=2) as osp,
            tc.tile_pool(name="gs", bufs=8) as gsp,
            tc.tile_pool(name="bank", bufs=6, space="PSUM") as bankp,
            tc.tile_pool(name="pse", bufs=2, space="PSUM") as psep,
        ):
            wqk_sb = constp.tile([128, 2, 2 * D], BF16)
            wv_sb = constp.tile([128, 2, C], BF16)
            bqk_sb = constp.tile([2 * D, 1], F32)
            eye_sb = constp.tile([128, 128], BF16)
            negi4_sb = constp.tile([128, 512], BF16)
            nc.sync.dma_start(wqk_sb[:], wqk_d.rearrange("c p d -> p c d"))
            nc.sync.dma_start(wv_sb[:], wv_d.rearrange("c p d -> p c d"))
            nc.sync.dma_start(bqk_sb[:], bqk_d)
            nc.sync.dma_start(eye_sb[:], eye_d)
            nc.sync.dma_start(negi4_sb[:], negi4_d)

            # persistent V half-tile; 1/gamma column memset once
            vt = vresp.tile([128, WHALF, CU], BF16)
            nc.vector.memset(vt[:, :, C], RGAMMA)

            for bi in range(BPC):
                # ---- XT load: plain contiguous DMAs (host pre-transposed) ----
                xt = xtp.tile([128, 2, HWPIX], BF16)
                QT4 = HWPIX // 4
                for qq in range(4):
                    psl = slice(qq * QT4, (qq + 1) * QT4)
                    for cc in range(2):
                        nc.sync.dma_start(xt[:, cc, psl], xbf_t[bi, cc, :, psl])
                xtv = xt.rearrange("p c (h w) -> p c h w", h=H)

                # ---- packed q|k projection, bias added in the drains ----
                qt = qkp.tile([D, HWPIX], BF16, tag="qt")
                kt = qkp.tile([D, HWPIX], BF16, tag="kt")
                for pc in range(HWPIX // 512):
                    sl = slice(pc * 512, (pc + 1) * 512)
                    pq = bankp.tile([2 * D, 512], F32, tag="bank")
                    nc.tensor.matmul(pq[:], wqk_sb[:, 0, :], xt[:, 0, sl], start=True, stop=False)
                    nc.tensor.matmul(pq[:], wqk_sb[:, 1, :], xt[:, 1, sl], start=False, stop=True)
                    nc.vector.tensor_scalar_add(qt[:, sl], pq[0:D, :], bqk_sb[0:D, :])
                    nc.scalar.activation(kt[:, sl], pq[D : 2 * D, :], AF.Identity, bias=bqk_sb[D : 2 * D, :])
                qtv = qt.rearrange("p (h w) -> p h w", h=H)
                ktv = kt.rearrange("p (h w) -> p h w", h=H)

                uh_hw = uhv1_d[bi].rearrange("(w h) t c -> h w t c", h=H)

                # ---- phase A: per half -- V compute + column attention ----
                for hf in range(2):
                    wbase = hf * WHALF

                    # V for columns [wbase, wbase+64): 2 cols per PSUM bank
                    for wp in range(WHALF // 2):
                        w0 = wbase + 2 * wp
                        pv = bankp.tile([128, 2, C], F32, tag="bank")
                        for j in range(2):
                            for cc in range(2):
                                nc.tensor.matmul(
                                    pv[:, j, :], xtv[:, cc, :, w0 + j], wv_sb[:, cc, :],
                                    start=(cc == 0), stop=(cc == 1),
                                )
                        dst = vt[:, 2 * wp : 2 * wp + 2, :C]
                        if wp % 2 == 0:
                            nc.scalar.activation(dst, pv[:], AF.Copy)
                        else:
                            nc.vector.tensor_copy(dst, pv[:])
                        if wp % 4 == 3:
                            # scatter 8 cols of V to the w-major scratch
                            wsl = slice(2 * wp - 6, 2 * wp + 2)
                            gsl = slice(wbase + 2 * wp - 6, wbase + 2 * wp + 2)
                            nc.sync.dma_start(uh_hw[:, gsl, 1, :], vt[:, wsl, :])

                    # column attention, software-pipelined depth 2
                    NGA = WHALF // 4
                    ex_q = []
                    ust = None
                    for g in range(NGA + 2):
                        if g < NGA:
                            w4 = wbase + g * 4
                            pe4 = psep.tile([128, 4, 128], F32, tag="pe")
                            for i in range(4):
                                nc.tensor.matmul(
                                    pe4[:, i, :], ktv[:, :, w4 + i], qtv[:, :, w4 + i],
                                    start=(i == 0), stop=False, skip_group_check=True,
                                )
                            nc.tensor.matmul(
                                pe4.rearrange("p a b -> p (a b)"), eye_sb[:], negi4_sb[:],
                                start=False, stop=True, skip_group_check=True,
                            )
                            ex4 = ep.tile([128, 4, 128], BF16, tag="ex")
                            nc.scalar.activation(ex4[:], pe4[:], AF.Exp)
                            ex_q.append(ex4)
                        if g >= 2:
                            gg = g - 2
                            exg = ex_q[gg]
                            if gg % 2 == 0:
                                ust = usp.tile([128, 8, CU], BF16, tag="ust")
                            for i in range(4):
                                wl = gg * 4 + i  # w local to the half-image
                                pu = bankp.tile([128, 512], F32, tag="bank")
                                nc.tensor.matmul(
                                    pu[:, :CU], exg[:, i, :], vt[:, wl, :],
                                    start=True, stop=True,
                                )
                                dst = ust[:, (gg % 2) * 4 + i, :]
                                if i % 2 == 0:
                                    nc.vector.tensor_copy(dst, pu[:, :CU])
                                else:
                                    nc.scalar.activation(dst, pu[:, :CU], AF.Copy)
                            if gg % 2 == 1:
                                wsl = slice(wbase + (gg - 1) * 4, wbase + (gg + 1) * 4)
                                nc.sync.dma_start(uh_hw[:, wsl, 0, :], ust[:])

                # ---- phase B: row attention + merge + fused epilogue ----
                uhv1_w = uhv1_d[bi].rearrange("(w h) t c -> w h t c", w=W)
                xr_w = xres_w[bi].rearrange("(w h) c -> w h c", w=W)
                out_w = out_d[bi].rearrange("(w h) c -> w h c", w=W)

                NB = H // 8  # 8-row DMA blocks
                ulvl_t = {}
                xr_t = {}

                def issue_loads(b):
                    hsl = slice(b * 8, (b + 1) * 8)
                    ul = ulp.tile([128, 8, 2, CU], BF16, tag="ulvl", name=f"ulvl{b}")
                    nc.sync.dma_start(ul[:], uhv1_w[:, hsl, :, :])
                    xr = xrp.tile([128, 8, C], BF16, tag="xr", name=f"xr{b}")
                    nc.sync.dma_start(xr[:], xr_w[:, hsl, :])
                    ulvl_t[b] = ul
                    xr_t[b] = xr

                issue_loads(0)
                issue_loads(1)

                DB = 3  # phase-B pipeline depth
                NGB = H // 4
                ex_q = []
                for g in range(NGB + DB):
                    if g < NGB:
                        if g % 2 == 0:
                            b = g // 2 + 2
                            if b < NB:
                                issue_loads(b)
                        h4 = g * 4
                        pe4 = psep.tile([128, 4, 128], F32, tag="pe")
                        for i in range(4):
                            nc.tensor.matmul(
                                pe4[:, i, :], ktv[:, h4 + i, :], qtv[:, h4 + i, :],
                                start=(i == 0), stop=(i == 3), skip_group_check=True,
                            )
                        ex4 = ep.tile([128, 4, 128], BF16, tag="ex")
                        nc.scalar.activation(ex4[:], pe4[:], AF.Exp)
                        ex_q.append(ex4)
                    if g >= DB:
                        gg = g - DB
                        exg = ex_q[gg]
                        b = gg // 2
                        ul = ulvl_t[b]
                        xr = xr_t[b]
                        ost = osp.tile([128, 4, C], BF16, tag="ost", name=f"ost{gg}", bufs=3)
                        pus = []
                        for i in range(4):
                            r = (gg % 2) * 4 + i  # row within the block
                            pu = bankp.tile([128, 512], F32, tag="bank")
                            nc.tensor.matmul(
                                pu[:, :CU], exg[:, i, :], ul[:, r, 1, :],
                                start=True, stop=False, skip_group_check=True,
                            )
                            nc.tensor.matmul(
                                pu[:, :CU], eye_sb[:], ul[:, r, 0, :],
                                start=False, stop=True, skip_group_check=True,
                            )
                            pus.append(pu)
                        # reciprocals first: shortens the PSUM-free chain
                        gss = []
                        for i in range(4):
                            gs2 = gsp.tile([128, 1], F32, tag="gs")
                            nc.vector.reciprocal(gs2, pus[i][:, C : C + 1])
                            gss.append(gs2)
                        for i in range(4):
                            r = (gg % 2) * 4 + i
                            # keep vector recip-only in phase B: scalar scales
                            # out of PSUM (freeing the bank), gpsimd adds xres
                            r2 = gsp.tile([128, C], BF16, tag="r2", bufs=4)
                            nc.scalar.activation(r2, pus[i][:, :C], AF.Copy, scale=gss[i])
                            nc.gpsimd.tensor_add(ost[:, i, :], r2, xr[:, r, :])
                        hsl = slice(gg * 4, (gg + 1) * 4)
                        nc.gpsimd.dma_start(out_w[:, hsl, :], ost[:])

    nc.compile()
    return nc


_NC_CACHE = None


def _get_nc():
    global _NC_CACHE
    if _NC_CACHE is None:
        _NC_CACHE = build_program()
    return _NC_CACHE


def make_in_maps(x, wq, bq, wk, bk, wv, bv):
    bf = ml_dtypes.bfloat16
    x = np.asarray(x, np.float32)
    # channel-major (transposed) x, h-major pixel index
    xbf_t_full = np.ascontiguousarray(
        x.reshape(B, HWPIX, C).transpose(0, 2, 1)
    ).astype(bf).reshape(B, 2, 128, HWPIX)
    # residual source in w-major pixel order
    xres_w_full = np.ascontiguousarray(
        (x + GAMMA * np.asarray(bv, np.float32)).transpose(0, 2, 1, 3)
    ).astype(bf).reshape(B, HWPIX, C)

    wqk = np.concatenate(
        [np.asarray(wq, np.float32), np.asarray(wk, np.float32)], axis=1
    ).astype(bf).reshape(2, 128, 2 * D)
    bqk = np.concatenate(
        [np.asarray(bq, np.float32), np.asarray(bk, np.float32)]
    ).reshape(2 * D, 1).astype(np.float32)
    wv_b = np.asarray(wv, np.float32).astype(bf).reshape(2, 128, C)
    eye = np.eye(128, dtype=bf)
    negi4 = np.tile((NEGBIG * np.eye(128, dtype=np.float32)).astype(bf), (1, 4))

    in_maps = []
    for ci in range(NCORES):
        sl = slice(ci * BPC, (ci + 1) * BPC)
        in_maps.append(
            {
                "xbf_t": xbf_t_full[sl],
                "xres_w": xres_w_full[sl],
                "wqk_b": wqk,
                "wv_b": wv_b,
                "bqk_f": bqk,
                "eye_b": eye,
                "negi4_b": negi4,
            }
        )
    return in_maps


def kernel(x, wq, bq, wk, bk, wv, bv):
    in_maps = make_in_maps(x, wq, bq, wk, bk, wv, bv)
    nc = _get_nc()
    res = run_bass_kernel_spmd(nc, in_maps, core_ids=list(range(NCORES)))
    outs = []
    for ci in range(NCORES):
        o = np.asarray(res.results[ci]["out"], dtype=np.float32)
        outs.append(o.reshape(BPC, W, H, C).transpose(0, 2, 1, 3))
    return np.concatenate(outs, axis=0)
